# revision 33
# baseline (speedup 1.0000x reference)
"""VQ codebook (vq_codebook) Trainium2 kernel, 8-core data-parallel.

reference semantics (jax fp32, replicated exactly):
    zf = z.reshape(-1, 256)                                # [N=16384, 256]
    d = ||zf||^2 + ||emb||^2 - 2 zf @ emb.T                # [N, 8192] fp32
    idx = argmin(d, axis=1)  (first-index tie-break)
    z_q = emb[idx]; loss = (1 + 0.25) * mean((z_q - z)^2)
    z_q_st = z + (z_q - z)

Key facts this kernel relies on (validated on hardware):
  * ||emb_k||^2 <= 256/8192^2 = 3.8e-6 < ulp(||z||^2)/2 for all realistic
    ||z||^2 (chi^2_256 >> 128), so the reference's `A + B` rounds back to A
    exactly: d == fp32(A - fp32(2*(zf@emb.T))).  The kernel computes
    d = Relu(psum*scale + A) on the Scalar engine, reproducing the
    reference's elementwise fp32 rounding bit-exactly.
  * The matmul runs as a 3-term fp16 split at 1 cyc/row (vs fp32's ~7):
    z = zh + zl (fp16 head + fp16 residual), es = emb.T*2^15 = eh + el;
    psum = zh@eh + zl@eh + zh@el accumulated in fp32 PSUM.  Dot error
    ~1e-9 (on par with HW fp32 matmul; fp32 d quantum is 3.05e-5), so the
    fp32-rounded d and its argmin ties match the reference exactly.
    The PE computes fp16 subnormal products exactly (measured) so the
    tiny zl/el residuals are safe.
  * nc.vector.max_index matches values by equality, returning the FIRST
    occurrence -- identical tie-breaking to jnp.argmin.

Schedule: per 128-token tile, matmuls run weight-major over 4-chunk PSUM
groups (one LDWEIGHTS per pass, 4 streaming matmuls); the Scalar engine
drains PSUM into the d tile; the Vector engine does one full-row
reduce_min and one full-row max_index (the DVE floor: 1 elem/cycle).
Gather/straight-through/loss work is deferred to a tail phase so the ACT
in-order queue never blocks PSUM recycling on DVE results.

Sharding: z split into 8 token shards of 2048; emb (fp32 for the gather,
fp16 split halves for the matmul) replicated per core; loss partial sums
per core combined on host.
"""

import numpy as np

import concourse.bass as bass
import concourse.mybir as mybir
from concourse import bacc, bass_utils
from concourse.tile import TileContext

NCORES = 8
B, T, D = 16, 1024, 256
K = 8192
NTOK = B * T                 # 16384
SHARD = NTOK // NCORES       # 2048
P = 128                      # partitions / tokens per tile
NTILES = SHARD // P          # 16
CHUNK = 512                  # codes per PSUM chunk
NCH = K // CHUNK             # 16
BETA = 0.25

_cache = {}

# matmul path: "f32" (HW fp32, 4+ cyc/row) or "f16split" (3x fp16
# matmuls: zh@eh + zl@eh + zh@el, emb pre-scaled 2^15; ~5e-10 dot err)
MM_MODE = "f16split"
ESCALE = 15  # emb scaled by 2^15 host-side (2x fold + 2^14 fp16-range shift)


def _build(reps=1, mm_mode=None):
    mm_mode = mm_mode or MM_MODE
    nc = bacc.Bacc("TRN2", target_bir_lowering=False, debug=False)
    z_d = nc.dram_tensor("z", [SHARD, D], mybir.dt.float32, kind="ExternalInput").ap()
    emb_d = nc.dram_tensor("emb", [K, D], mybir.dt.float32, kind="ExternalInput").ap()
    if mm_mode == "f32":
        zT_d = nc.dram_tensor("zt", [D, SHARD], mybir.dt.float32, kind="ExternalInput").ap()
        embT2_d = nc.dram_tensor("embt2", [D, K], mybir.dt.float32, kind="ExternalInput").ap()
    else:
        zh_d = nc.dram_tensor("zh", [D, SHARD], mybir.dt.float16, kind="ExternalInput").ap()
        zl_d = nc.dram_tensor("zl", [D, SHARD], mybir.dt.float16, kind="ExternalInput").ap()
        eh_d = nc.dram_tensor("eh", [D, K], mybir.dt.float16, kind="ExternalInput").ap()
        el_d = nc.dram_tensor("el", [D, K], mybir.dt.float16, kind="ExternalInput").ap()
    zqst_d = nc.dram_tensor("zqst", [SHARD, D], mybir.dt.float32, kind="ExternalOutput").ap()
    idx_d = nc.dram_tensor("idx", [SHARD], mybir.dt.int32, kind="ExternalOutput").ap()
    loss_d = nc.dram_tensor("losspart", [P, NTILES], mybir.dt.float32, kind="ExternalOutput").ap()

    with TileContext(nc) as tc:
        with tc.tile_pool(name="embp", bufs=1) as embp, \
             tc.tile_pool(name="work", bufs=3) as work, \
             tc.tile_pool(name="dpool", bufs=2) as dpool, \
             tc.tile_pool(name="small", bufs=3) as small, \
             tc.tile_pool(name="tailp", bufs=16) as tailp, \
             tc.tile_pool(name="psum", bufs=2, space="PSUM") as psum_pool:

            # codebook (transposed, host-prescaled) stationary in SBUF
            pre = {}

            def _preload(t):
                tok = slice(t * P, (t + 1) * P)
                ztile = work.tile([P, D], mybir.dt.float32, tag="ztile")
                nc.sync.dma_start(out=ztile, in_=z_d[tok, :])
                zhT = work.tile([P, 2, P], mybir.dt.float16, tag="zhT")
                nc.sync.dma_start(
                    out=zhT, in_=zh_d[:, tok].rearrange("(h p) m -> p h m", p=P))
                zlT = work.tile([P, 2, P], mybir.dt.float16, tag="zlT")
                nc.sync.dma_start(
                    out=zlT, in_=zl_d[:, tok].rearrange("(h p) m -> p h m", p=P))
                pre[t] = (ztile, zhT, zlT)

            if mm_mode == "f32":
                et = embp.tile([P, 2, K], mybir.dt.float32, tag="et")
                nc.sync.dma_start(out=et, in_=embT2_d.rearrange("(h p) k -> p h k", p=P))
                d_scale = -1.0
            else:
                NSEG = 4
                SEG = K // NSEG
                ehs, els = [], []

                def _load_seg(lst, src_d, s, nm):
                    ks = slice(s * SEG, (s + 1) * SEG)
                    seg = embp.tile([P, 2, SEG], mybir.dt.float16,
                                    name=f"{nm}{s}", tag=f"{nm}{s}")
                    nc.sync.dma_start(
                        out=seg, in_=src_d[:, ks].rearrange("(h p) k -> p h k", p=P))
                    lst.append(seg)

                # issue order = first-use order: eh0 + tile-0 z, el0,
                # tiles 1-2 z, then the remaining segments
                _load_seg(ehs, eh_d, 0, "eh")
                _preload(0)
                _load_seg(els, el_d, 0, "el")
                _preload(1)
                _preload(2)
                for s in range(1, NSEG):
                    _load_seg(ehs, eh_d, s, "eh")
                    _load_seg(els, el_d, s, "el")
                d_scale = -float(2.0 ** (-(ESCALE - 1)))  # undo 2^15, keep 2x

            lp_all = embp.tile([P, NTILES], mybir.dt.float32, tag="lp")
            w8all = embp.tile([P, NTILES], mybir.dt.uint32, tag="w8all")

            def tail_work(t):
                # gather z_q, straight-through output, loss partial for tile t.
                # Emitted 2 tiles behind the main work so every dependency
                # (FIND -> w8all -> gather) is already satisfied when the
                # in-order ACT/DVE queues reach these ops.
                tok = slice(t * P, (t + 1) * P)
                zt2 = tailp.tile([P, D], mybir.dt.float32, tag="zt2")
                nc.sync.dma_start(out=zt2, in_=z_d[tok, :])
                zq = tailp.tile([P, D], mybir.dt.float32, tag="zq")
                nc.gpsimd.indirect_dma_start(
                    out=zq, out_offset=None, in_=emb_d,
                    in_offset=bass.IndirectOffsetOnAxis(ap=w8all[:, t:t + 1], axis=0))
                diff = work.tile([P, D], mybir.dt.float32, tag="diff")
                nc.vector.tensor_sub(out=diff, in0=zq, in1=zt2)
                st = work.tile([P, D], mybir.dt.float32, tag="st")
                nc.vector.tensor_add(out=st, in0=zt2, in1=diff)
                nc.sync.dma_start(out=zqst_d[tok, :], in_=st)
                sq2 = work.tile([P, D], mybir.dt.float32, tag="sq2x")
                nc.scalar.activation(out=sq2, in_=diff,
                                     func=mybir.ActivationFunctionType.Square,
                                     accum_out=lp_all[:, t:t + 1])

            def body():
                for t in range(NTILES):
                    tok = slice(t * P, (t + 1) * P)
                    if mm_mode == "f32":
                        ztile = work.tile([P, D], mybir.dt.float32, tag="ztile")
                        nc.sync.dma_start(out=ztile, in_=z_d[tok, :])
                        ztT = work.tile([P, 2, P], mybir.dt.float32, tag="ztT")
                        nc.sync.dma_start(
                            out=ztT, in_=zT_d[:, tok].rearrange("(h p) m -> p h m", p=P))
                    elif t in pre:
                        ztile, zhT, zlT = pre[t]
                    else:
                        ztile = work.tile([P, D], mybir.dt.float32, tag="ztile")
                        nc.sync.dma_start(out=ztile, in_=z_d[tok, :])
                        zhT = work.tile([P, 2, P], mybir.dt.float16, tag="zhT")
                        nc.sync.dma_start(
                            out=zhT, in_=zh_d[:, tok].rearrange("(h p) m -> p h m", p=P))
                        zlT = work.tile([P, 2, P], mybir.dt.float16, tag="zlT")
                        nc.sync.dma_start(
                            out=zlT, in_=zl_d[:, tok].rearrange("(h p) m -> p h m", p=P))

                    # A = sum(z^2) per token
                    sq = work.tile([P, D], mybir.dt.float32, tag="sq")
                    Atile = small.tile([P, 1], mybir.dt.float32, tag="A")
                    nc.scalar.activation(out=sq, in_=ztile,
                                         func=mybir.ActivationFunctionType.Square,
                                         accum_out=Atile)

                    dtile = dpool.tile([P, K], mybir.dt.float32, tag="d")
                    if mm_mode == "f32":
                        passes = [(ztT, [et])]
                    else:
                        passes = [(zhT, ehs), (zlT, ehs), (zhT, els)]
                    # weight-major over groups of 8 chunks: each (w, h) pass
                    # loads weights once and streams 8 same-weight matmuls
                    GRP = 4
                    for g in range(NCH // GRP):
                        pss = []
                        for j in range(GRP):
                            ps_j = psum_pool.tile([P, CHUNK], mybir.dt.float32,
                                                  name=f"ps{j}", tag=f"ps{j}")
                            pss.append(ps_j)
                        np_ = len(passes)
                        for i, (w, segs) in enumerate(passes):
                            for h in range(2):
                                for j in range(GRP):
                                    c = g * GRP + j
                                    if len(segs) == 1:
                                        e, cc = segs[0], c
                                    else:
                                        e, cc = segs[c // GRP], c % GRP
                                    cs = slice(cc * CHUNK, (cc + 1) * CHUNK)
                                    nc.tensor.matmul(
                                        pss[j], lhsT=w[:, h, :], rhs=e[:, h, cs],
                                        start=(i == 0 and h == 0),
                                        stop=(i == np_ - 1 and h == 1))
                        for j in range(GRP):
                            c = g * GRP + j
                            cs = slice(c * CHUNK, (c + 1) * CHUNK)
                            # d = Relu(A - 2C); Relu is identity: d >> 0
                            nc.scalar.activation(
                                out=dtile[:, cs], in_=pss[j],
                                func=mybir.ActivationFunctionType.Relu,
                                bias=Atile, scale=d_scale)

                    gmin = small.tile([P, 1], mybir.dt.float32, tag="gmin")
                    nc.vector.tensor_reduce(out=gmin, in_=dtile,
                                            axis=mybir.AxisListType.X,
                                            op=mybir.AluOpType.min)
                    gmin8 = small.tile([P, 8], mybir.dt.float32, tag="gmin8")
                    nc.vector.tensor_copy(out=gmin8, in_=gmin.to_broadcast([P, 8]))
                    w8 = small.tile([P, 8], mybir.dt.uint32, tag="w8")
                    nc.vector.max_index(out=w8, in_max=gmin8, in_values=dtile)
                    nc.vector.tensor_copy(out=w8all[:, t:t + 1], in_=w8[:, 0:1])
                    nc.sync.dma_start(
                        out=idx_d[tok].rearrange("(p one) -> p one", one=1),
                        in_=w8[:, 0:1].bitcast(mybir.dt.int32))
                    if t >= 2:
                        tail_work(t - 2)
                for t in range(max(0, NTILES - 2), NTILES):
                    tail_work(t)



            if reps == 1:
                body()
            else:
                with tc.For_i(0, reps, 1):
                    body()

            nc.sync.dma_start(out=loss_d, in_=lp_all)

    nc.compile()
    return nc


def _in_maps(z, emb, mm_mode=None):
    mm_mode = mm_mode or MM_MODE
    zf = np.ascontiguousarray(z.reshape(NTOK, D))
    emb_c = np.ascontiguousarray(emb)
    in_maps = []
    if mm_mode == "f32":
        embT2 = np.ascontiguousarray((2.0 * emb).T)
        for c in range(NCORES):
            zs = np.ascontiguousarray(zf[c * SHARD:(c + 1) * SHARD])
            in_maps.append({
                "z": zs,
                "zt": np.ascontiguousarray(zs.T),
                "embt2": embT2,
                "emb": emb_c,
            })
        return in_maps

    import ml_dtypes
    bf16 = ml_dtypes.bfloat16
    # emb side: es = emb.T * 2^15 (2x fold + fp16-range shift);
    # eh = fp16(es) [11-bit head], el = bf16(es - eh) [cross term]
    es = emb.T.astype(np.float32) * np.float32(2.0 ** ESCALE)
    eh = es.astype(np.float16)
    el = (es - eh.astype(np.float32)).astype(np.float16)
    eh = np.ascontiguousarray(eh)
    el = np.ascontiguousarray(el)
    # z side: zh = fp16(zT) [11-bit head], zl = fp16(zT - zh)
    # (PE computes fp16 subnormals exactly -- verified on HW)
    zT = zf.T.astype(np.float32)
    zh = zT.astype(np.float16)
    zl = (zT - zh.astype(np.float32)).astype(np.float16)
    for c in range(NCORES):
        tok = slice(c * SHARD, (c + 1) * SHARD)
        in_maps.append({
            "z": np.ascontiguousarray(zf[tok]),
            "zh": np.ascontiguousarray(zh[:, tok]),
            "zl": np.ascontiguousarray(zl[:, tok]),
            "eh": eh,
            "el": el,
            "emb": emb_c,
        })
    return in_maps


def _assemble(results):
    zqst = np.concatenate([r["zqst"] for r in results], axis=0).reshape(B, T, D)
    idx = np.concatenate([r["idx"] for r in results], axis=0).astype(np.int32)
    total = np.sum([r["losspart"].astype(np.float64).sum() for r in results])
    m = np.float32(total / (NTOK * D))
    loss = np.float32(m + np.float32(BETA) * m)
    return zqst, idx, loss


def _run(z, emb):
    if "nc" not in _cache:
        _cache["nc"] = _build()
    res = bass_utils.run_bass_kernel_spmd(
        _cache["nc"], in_maps=_in_maps(z, emb), core_ids=list(range(NCORES)))
    return _assemble(res.results)


def kernel(z, emb):
    return _run(np.asarray(z), np.asarray(emb))


# revision 35
# speedup vs baseline: 1.0019x; 1.0019x over previous
"""VQ codebook (vq_codebook) Trainium2 kernel, 8-core data-parallel.

reference semantics (jax fp32, replicated exactly):
    zf = z.reshape(-1, 256)                                # [N=16384, 256]
    d = ||zf||^2 + ||emb||^2 - 2 zf @ emb.T                # [N, 8192] fp32
    idx = argmin(d, axis=1)  (first-index tie-break)
    z_q = emb[idx]; loss = (1 + 0.25) * mean((z_q - z)^2)
    z_q_st = z + (z_q - z)

Key facts this kernel relies on (validated on hardware):
  * ||emb_k||^2 <= 256/8192^2 = 3.8e-6 < ulp(||z||^2)/2 for all realistic
    ||z||^2 (chi^2_256 >> 128), so the reference's `A + B` rounds back to A
    exactly: d == fp32(A - fp32(2*(zf@emb.T))).  The kernel computes
    d = Relu(psum*scale + A) on the Scalar engine, reproducing the
    reference's elementwise fp32 rounding bit-exactly.
  * The matmul runs as a 3-term fp16 split at 1 cyc/row (vs fp32's ~7):
    z = zh + zl (fp16 head + fp16 residual), es = emb.T*2^15 = eh + el;
    psum = zh@eh + zl@eh + zh@el accumulated in fp32 PSUM.  Dot error
    ~1e-9 (on par with HW fp32 matmul; fp32 d quantum is 3.05e-5), so the
    fp32-rounded d and its argmin ties match the reference exactly.
    The PE computes fp16 subnormal products exactly (measured) so the
    tiny zl/el residuals are safe.
  * nc.vector.max_index matches values by equality, returning the FIRST
    occurrence -- identical tie-breaking to jnp.argmin.

Schedule: per 128-token tile, matmuls run weight-major over 4-chunk PSUM
groups (one LDWEIGHTS per pass, 4 streaming matmuls); the Scalar engine
drains PSUM into the d tile; the Vector engine does one full-row
reduce_min and one full-row max_index (the DVE floor: 1 elem/cycle).
Gather/straight-through/loss work is deferred to a tail phase so the ACT
in-order queue never blocks PSUM recycling on DVE results.

Sharding: z split into 8 token shards of 2048; emb (fp32 for the gather,
fp16 split halves for the matmul) replicated per core; loss partial sums
per core combined on host.
"""

import numpy as np

import concourse.bass as bass
import concourse.mybir as mybir
from concourse import bacc, bass_utils
from concourse.tile import TileContext

NCORES = 8
B, T, D = 16, 1024, 256
K = 8192
NTOK = B * T                 # 16384
SHARD = NTOK // NCORES       # 2048
P = 128                      # partitions / tokens per tile
NTILES = SHARD // P          # 16
CHUNK = 512                  # codes per PSUM chunk
NCH = K // CHUNK             # 16
BETA = 0.25

_cache = {}

# matmul path: "f32" (HW fp32, 4+ cyc/row) or "f16split" (3x fp16
# matmuls: zh@eh + zl@eh + zh@el, emb pre-scaled 2^15; ~5e-10 dot err)
MM_MODE = "f16split"
ESCALE = 15  # emb scaled by 2^15 host-side (2x fold + 2^14 fp16-range shift)


def _build(reps=1, mm_mode=None):
    mm_mode = mm_mode or MM_MODE
    nc = bacc.Bacc("TRN2", target_bir_lowering=False, debug=False)
    z_d = nc.dram_tensor("z", [SHARD, D], mybir.dt.float32, kind="ExternalInput").ap()
    emb_d = nc.dram_tensor("emb", [K, D], mybir.dt.float32, kind="ExternalInput").ap()
    if mm_mode == "f32":
        zT_d = nc.dram_tensor("zt", [D, SHARD], mybir.dt.float32, kind="ExternalInput").ap()
        embT2_d = nc.dram_tensor("embt2", [D, K], mybir.dt.float32, kind="ExternalInput").ap()
    else:
        zh_d = nc.dram_tensor("zh", [D, SHARD], mybir.dt.float16, kind="ExternalInput").ap()
        zl_d = nc.dram_tensor("zl", [D, SHARD], mybir.dt.float16, kind="ExternalInput").ap()
        eh_d = nc.dram_tensor("eh", [D, K], mybir.dt.float16, kind="ExternalInput").ap()
        el_d = nc.dram_tensor("el", [D, K], mybir.dt.float16, kind="ExternalInput").ap()
    zqst_d = nc.dram_tensor("zqst", [SHARD, D], mybir.dt.float32, kind="ExternalOutput").ap()
    idx_d = nc.dram_tensor("idx", [SHARD], mybir.dt.int32, kind="ExternalOutput").ap()
    loss_d = nc.dram_tensor("losspart", [P, NTILES], mybir.dt.float32, kind="ExternalOutput").ap()

    with TileContext(nc) as tc:
        with tc.tile_pool(name="embp", bufs=1) as embp, \
             tc.tile_pool(name="work", bufs=3) as work, \
             tc.tile_pool(name="dpool", bufs=2) as dpool, \
             tc.tile_pool(name="small", bufs=3) as small, \
             tc.tile_pool(name="tailp", bufs=16) as tailp, \
             tc.tile_pool(name="psum", bufs=2, space="PSUM") as psum_pool:

            # codebook (transposed, host-prescaled) stationary in SBUF
            pre = {}

            def _preload(t):
                tok = slice(t * P, (t + 1) * P)
                ztile = work.tile([P, D], mybir.dt.float32, tag="ztile")
                nc.sync.dma_start(out=ztile, in_=z_d[tok, :])
                zhT = work.tile([P, 2, P], mybir.dt.float16, tag="zhT")
                nc.sync.dma_start(
                    out=zhT, in_=zh_d[:, tok].rearrange("(h p) m -> p h m", p=P))
                zlT = work.tile([P, 2, P], mybir.dt.float16, tag="zlT")
                nc.sync.dma_start(
                    out=zlT, in_=zl_d[:, tok].rearrange("(h p) m -> p h m", p=P))
                pre[t] = (ztile, zhT, zlT)

            if mm_mode == "f32":
                et = embp.tile([P, 2, K], mybir.dt.float32, tag="et")
                nc.sync.dma_start(out=et, in_=embT2_d.rearrange("(h p) k -> p h k", p=P))
                d_scale = -1.0
            else:
                NSEG = 4
                SEG = K // NSEG
                ehs, els = [], []

                def _load_seg(lst, src_d, s, nm):
                    ks = slice(s * SEG, (s + 1) * SEG)
                    seg = embp.tile([P, 2, SEG], mybir.dt.float16,
                                    name=f"{nm}{s}", tag=f"{nm}{s}")
                    nc.sync.dma_start(
                        out=seg, in_=src_d[:, ks].rearrange("(h p) k -> p h k", p=P))
                    lst.append(seg)

                # issue order = first-use order: eh0 + tile-0 z, el0,
                # tiles 1-2 z, then the remaining segments
                _load_seg(ehs, eh_d, 0, "eh")
                _preload(0)
                _load_seg(els, el_d, 0, "el")
                _preload(1)
                _preload(2)
                for s in range(1, NSEG):
                    _load_seg(ehs, eh_d, s, "eh")
                    _load_seg(els, el_d, s, "el")
                d_scale = -float(2.0 ** (-(ESCALE - 1)))  # undo 2^15, keep 2x

            lp_all = embp.tile([P, NTILES], mybir.dt.float32, tag="lp")
            w8all = embp.tile([P, NTILES], mybir.dt.uint32, tag="w8all")

            def tail_work(t):
                # gather z_q, straight-through output, loss partial for tile t.
                # Emitted 2 tiles behind the main work so every dependency
                # (FIND -> w8all -> gather) is already satisfied when the
                # in-order ACT/DVE queues reach these ops.
                tok = slice(t * P, (t + 1) * P)
                zt2 = tailp.tile([P, D], mybir.dt.float32, tag="zt2")
                nc.sync.dma_start(out=zt2, in_=z_d[tok, :])
                zq = tailp.tile([P, D], mybir.dt.float32, tag="zq")
                nc.gpsimd.indirect_dma_start(
                    out=zq, out_offset=None, in_=emb_d,
                    in_offset=bass.IndirectOffsetOnAxis(ap=w8all[:, t:t + 1], axis=0))
                diff = work.tile([P, D], mybir.dt.float32, tag="diff")
                nc.vector.tensor_sub(out=diff, in0=zq, in1=zt2)
                st = work.tile([P, D], mybir.dt.float32, tag="st")
                nc.vector.tensor_add(out=st, in0=zt2, in1=diff)
                nc.sync.dma_start(out=zqst_d[tok, :], in_=st)
                sq2 = work.tile([P, D], mybir.dt.float32, tag="sq2x")
                nc.scalar.activation(out=sq2, in_=diff,
                                     func=mybir.ActivationFunctionType.Square,
                                     accum_out=lp_all[:, t:t + 1])

            def body():
                for t in range(NTILES):
                    tok = slice(t * P, (t + 1) * P)
                    if mm_mode == "f32":
                        ztile = work.tile([P, D], mybir.dt.float32, tag="ztile")
                        nc.sync.dma_start(out=ztile, in_=z_d[tok, :])
                        ztT = work.tile([P, 2, P], mybir.dt.float32, tag="ztT")
                        nc.sync.dma_start(
                            out=ztT, in_=zT_d[:, tok].rearrange("(h p) m -> p h m", p=P))
                    elif t in pre:
                        ztile, zhT, zlT = pre[t]
                    else:
                        ztile = work.tile([P, D], mybir.dt.float32, tag="ztile")
                        nc.sync.dma_start(out=ztile, in_=z_d[tok, :])
                        zhT = work.tile([P, 2, P], mybir.dt.float16, tag="zhT")
                        nc.sync.dma_start(
                            out=zhT, in_=zh_d[:, tok].rearrange("(h p) m -> p h m", p=P))
                        zlT = work.tile([P, 2, P], mybir.dt.float16, tag="zlT")
                        nc.sync.dma_start(
                            out=zlT, in_=zl_d[:, tok].rearrange("(h p) m -> p h m", p=P))

                    # A = sum(z^2) per token
                    sq = work.tile([P, D], mybir.dt.float32, tag="sq")
                    Atile = small.tile([P, 1], mybir.dt.float32, tag="A")
                    nc.scalar.activation(out=sq, in_=ztile,
                                         func=mybir.ActivationFunctionType.Square,
                                         accum_out=Atile)

                    dtile = dpool.tile([P, K], mybir.dt.float32, tag="d")
                    if mm_mode == "f32":
                        passes = [(ztT, [et])]
                    else:
                        passes = [(zhT, ehs), (zlT, ehs), (zhT, els)]
                    # weight-major over groups of 8 chunks: each (w, h) pass
                    # loads weights once and streams 8 same-weight matmuls
                    GRP = 4
                    for g in range(NCH // GRP):
                        pss = []
                        for j in range(GRP):
                            ps_j = psum_pool.tile([P, CHUNK], mybir.dt.float32,
                                                  name=f"ps{j}", tag=f"ps{j}")
                            pss.append(ps_j)
                        np_ = len(passes)
                        for i, (w, segs) in enumerate(passes):
                            for h in range(2):
                                for j in range(GRP):
                                    c = g * GRP + j
                                    if len(segs) == 1:
                                        e, cc = segs[0], c
                                    else:
                                        e, cc = segs[c // GRP], c % GRP
                                    cs = slice(cc * CHUNK, (cc + 1) * CHUNK)
                                    nc.tensor.matmul(
                                        pss[j], lhsT=w[:, h, :], rhs=e[:, h, cs],
                                        start=(i == 0 and h == 0),
                                        stop=(i == np_ - 1 and h == 1))
                        for j in range(GRP):
                            c = g * GRP + j
                            cs = slice(c * CHUNK, (c + 1) * CHUNK)
                            # d = Relu(A - 2C); Relu is identity: d >> 0
                            nc.scalar.activation(
                                out=dtile[:, cs], in_=pss[j],
                                func=mybir.ActivationFunctionType.Relu,
                                bias=Atile, scale=d_scale)

                    gmin = small.tile([P, 1], mybir.dt.float32, tag="gmin")
                    nc.vector.tensor_reduce(out=gmin, in_=dtile,
                                            axis=mybir.AxisListType.X,
                                            op=mybir.AluOpType.min)
                    gmin8 = small.tile([P, 8], mybir.dt.float32, tag="gmin8")
                    nc.vector.tensor_copy(out=gmin8, in_=gmin.to_broadcast([P, 8]))
                    w8 = small.tile([P, 8], mybir.dt.uint32, tag="w8")
                    nc.vector.max_index(out=w8, in_max=gmin8, in_values=dtile)
                    nc.vector.tensor_copy(out=w8all[:, t:t + 1], in_=w8[:, 0:1])
                    nc.sync.dma_start(
                        out=idx_d[tok].rearrange("(p one) -> p one", one=1),
                        in_=w8[:, 0:1].bitcast(mybir.dt.int32))
                    if t >= 2:
                        tail_work(t - 2)
                for t in range(max(0, NTILES - 2), NTILES):
                    tail_work(t)



            if reps == 1:
                body()
            else:
                with tc.For_i(0, reps, 1):
                    body()

            nc.sync.dma_start(out=loss_d, in_=lp_all)

    nc.compile()
    return nc


def _in_maps(z, emb, mm_mode=None):
    mm_mode = mm_mode or MM_MODE
    zf = np.ascontiguousarray(z.reshape(NTOK, D))
    emb_c = np.ascontiguousarray(emb)
    in_maps = []
    if mm_mode == "f32":
        embT2 = np.ascontiguousarray((2.0 * emb).T)
        for c in range(NCORES):
            zs = np.ascontiguousarray(zf[c * SHARD:(c + 1) * SHARD])
            in_maps.append({
                "z": zs,
                "zt": np.ascontiguousarray(zs.T),
                "embt2": embT2,
                "emb": emb_c,
            })
        return in_maps

    import ml_dtypes
    bf16 = ml_dtypes.bfloat16
    # emb side: es = emb.T * 2^15 (2x fold + fp16-range shift);
    # eh = fp16(es) [11-bit head], el = bf16(es - eh) [cross term]
    es = emb.T.astype(np.float32) * np.float32(2.0 ** ESCALE)
    eh = es.astype(np.float16)
    el = (es - eh.astype(np.float32)).astype(np.float16)
    eh = np.ascontiguousarray(eh)
    el = np.ascontiguousarray(el)
    # z side: zh = fp16(zT) [11-bit head], zl = fp16(zT - zh)
    # (PE computes fp16 subnormals exactly -- verified on HW)
    zT = zf.T.astype(np.float32)
    zh = zT.astype(np.float16)
    zl = (zT - zh.astype(np.float32)).astype(np.float16)
    for c in range(NCORES):
        tok = slice(c * SHARD, (c + 1) * SHARD)
        in_maps.append({
            "z": np.ascontiguousarray(zf[tok]),
            "zh": np.ascontiguousarray(zh[:, tok]),
            "zl": np.ascontiguousarray(zl[:, tok]),
            "eh": eh,
            "el": el,
            "emb": emb_c,
        })
    return in_maps


def _assemble(results):
    zqst = np.concatenate([r["zqst"] for r in results], axis=0).reshape(B, T, D)
    idx = np.concatenate([r["idx"] for r in results], axis=0).astype(np.int32)
    total = np.sum([r["losspart"].astype(np.float64).sum() for r in results])
    m = np.float32(total / (NTOK * D))
    loss = np.float32(m + np.float32(BETA) * m)
    return zqst, idx, loss


def _run(z, emb):
    if "nc" not in _cache:
        _cache["nc"] = _build()
    res = bass_utils.run_bass_kernel_spmd(
        _cache["nc"], in_maps=_in_maps(z, emb), core_ids=list(range(NCORES)))
    return _assemble(res.results)


def kernel(z, emb):
    return _run(np.asarray(z), np.asarray(emb))


# revision 36
# speedup vs baseline: 1.0226x; 1.0206x over previous
"""VQ codebook (vq_codebook) Trainium2 kernel, 8-core data-parallel.

reference semantics (jax fp32, replicated exactly):
    zf = z.reshape(-1, 256)                                # [N=16384, 256]
    d = ||zf||^2 + ||emb||^2 - 2 zf @ emb.T                # [N, 8192] fp32
    idx = argmin(d, axis=1)  (first-index tie-break)
    z_q = emb[idx]; loss = (1 + 0.25) * mean((z_q - z)^2)
    z_q_st = z + (z_q - z)

Key facts this kernel relies on (validated on hardware):
  * ||emb_k||^2 <= 256/8192^2 = 3.8e-6 < ulp(||z||^2)/2 for all realistic
    ||z||^2 (chi^2_256 >> 128), so the reference's `A + B` rounds back to A
    exactly: d == fp32(A - fp32(2*(zf@emb.T))).  The kernel computes
    d = Relu(psum*scale + A) on the Scalar engine, reproducing the
    reference's elementwise fp32 rounding bit-exactly.
  * The matmul runs as a 3-term fp16 split at 1 cyc/row (vs fp32's ~7):
    z = zh + zl (fp16 head + fp16 residual), es = emb.T*2^15 = eh + el;
    psum = zh@eh + zl@eh + zh@el accumulated in fp32 PSUM.  Dot error
    ~1e-9 (on par with HW fp32 matmul; fp32 d quantum is 3.05e-5), so the
    fp32-rounded d and its argmin ties match the reference exactly.
    The PE computes fp16 subnormal products exactly (measured) so the
    tiny zl/el residuals are safe.
  * nc.vector.max_index matches values by equality, returning the FIRST
    occurrence -- identical tie-breaking to jnp.argmin.

Schedule: per 128-token tile, matmuls run weight-major over 4-chunk PSUM
groups (one LDWEIGHTS per pass, 4 streaming matmuls); the Scalar engine
drains PSUM into the d tile; the Vector engine does one full-row
reduce_min and one full-row max_index (the DVE floor: 1 elem/cycle).
Gather/straight-through/loss work is deferred to a tail phase so the ACT
in-order queue never blocks PSUM recycling on DVE results.

Sharding: z split into 8 token shards of 2048; emb (fp32 for the gather,
fp16 split halves for the matmul) replicated per core; loss partial sums
per core combined on host.
"""

import numpy as np

import concourse.bass as bass
import concourse.mybir as mybir
from concourse import bacc, bass_utils
from concourse.tile import TileContext

NCORES = 8
B, T, D = 16, 1024, 256
K = 8192
NTOK = B * T                 # 16384
SHARD = NTOK // NCORES       # 2048
P = 128                      # partitions / tokens per tile
NTILES = SHARD // P          # 16
CHUNK = 512                  # codes per PSUM chunk
NCH = K // CHUNK             # 16
BETA = 0.25

_cache = {}

# matmul path: "f32" (HW fp32, 4+ cyc/row) or "f16split" (3x fp16
# matmuls: zh@eh + zl@eh + zh@el, emb pre-scaled 2^15; ~5e-10 dot err)
MM_MODE = "f16split"
ESCALE = 15  # emb scaled by 2^15 host-side (2x fold + 2^14 fp16-range shift)


def _build(reps=1, mm_mode=None):
    mm_mode = mm_mode or MM_MODE
    nc = bacc.Bacc("TRN2", target_bir_lowering=False, debug=False)
    z_d = nc.dram_tensor("z", [SHARD, D], mybir.dt.float32, kind="ExternalInput").ap()
    emb_d = nc.dram_tensor("emb", [K, D], mybir.dt.float32, kind="ExternalInput").ap()
    if mm_mode == "f32":
        zT_d = nc.dram_tensor("zt", [D, SHARD], mybir.dt.float32, kind="ExternalInput").ap()
        embT2_d = nc.dram_tensor("embt2", [D, K], mybir.dt.float32, kind="ExternalInput").ap()
    else:
        zh_d = nc.dram_tensor("zh", [D, SHARD], mybir.dt.float16, kind="ExternalInput").ap()
        zl_d = nc.dram_tensor("zl", [D, SHARD], mybir.dt.float16, kind="ExternalInput").ap()
        eh_d = nc.dram_tensor("eh", [D, K], mybir.dt.float16, kind="ExternalInput").ap()
        el_d = nc.dram_tensor("el", [D, K], mybir.dt.float16, kind="ExternalInput").ap()
    zqst_d = nc.dram_tensor("zqst", [SHARD, D], mybir.dt.float32, kind="ExternalOutput").ap()
    idx_d = nc.dram_tensor("idx", [SHARD], mybir.dt.int32, kind="ExternalOutput").ap()
    loss_d = nc.dram_tensor("losspart", [P, NTILES], mybir.dt.float32, kind="ExternalOutput").ap()

    with TileContext(nc) as tc:
        with tc.tile_pool(name="embp", bufs=1) as embp, \
             tc.tile_pool(name="work", bufs=3) as work, \
             tc.tile_pool(name="dpool", bufs=2) as dpool, \
             tc.tile_pool(name="small", bufs=3) as small, \
             tc.tile_pool(name="tailp", bufs=16) as tailp, \
             tc.tile_pool(name="psum", bufs=2, space="PSUM") as psum_pool:

            # codebook (transposed, host-prescaled) stationary in SBUF
            pre = {}

            def _preload(t):
                tok = slice(t * P, (t + 1) * P)
                ztile = work.tile([P, D], mybir.dt.float32, tag="ztile")
                nc.sync.dma_start(out=ztile, in_=z_d[tok, :])
                zhT = work.tile([P, 2, P], mybir.dt.float16, tag="zhT")
                nc.sync.dma_start(
                    out=zhT, in_=zh_d[:, tok].rearrange("(h p) m -> p h m", p=P))
                zlT = work.tile([P, 2, P], mybir.dt.float16, tag="zlT")
                nc.sync.dma_start(
                    out=zlT, in_=zl_d[:, tok].rearrange("(h p) m -> p h m", p=P))
                pre[t] = (ztile, zhT, zlT)

            if mm_mode == "f32":
                et = embp.tile([P, 2, K], mybir.dt.float32, tag="et")
                nc.sync.dma_start(out=et, in_=embT2_d.rearrange("(h p) k -> p h k", p=P))
                d_scale = -1.0
            else:
                NSEG = 4
                SEG = K // NSEG
                ehs, els = [], []

                def _load_seg(lst, src_d, s, nm):
                    ks = slice(s * SEG, (s + 1) * SEG)
                    seg = embp.tile([P, 2, SEG], mybir.dt.float16,
                                    name=f"{nm}{s}", tag=f"{nm}{s}")
                    nc.sync.dma_start(
                        out=seg, in_=src_d[:, ks].rearrange("(h p) k -> p h k", p=P))
                    lst.append(seg)

                # issue order = first-use order: eh0 + tile-0 z, el0,
                # tiles 1-2 z, then the remaining segments
                _load_seg(ehs, eh_d, 0, "eh")
                _preload(0)
                _load_seg(els, el_d, 0, "el")
                _preload(1)
                _preload(2)
                for s in range(1, NSEG):
                    _load_seg(ehs, eh_d, s, "eh")
                    _load_seg(els, el_d, s, "el")
                d_scale = -float(2.0 ** (-(ESCALE - 1)))  # undo 2^15, keep 2x

            lp_all = embp.tile([P, NTILES], mybir.dt.float32, tag="lp")
            w8all = embp.tile([P, NTILES], mybir.dt.uint32, tag="w8all")

            def tail_work(t):
                # gather z_q, straight-through output, loss partial for tile t.
                # Emitted 2 tiles behind the main work so every dependency
                # (FIND -> w8all -> gather) is already satisfied when the
                # in-order ACT/DVE queues reach these ops.
                tok = slice(t * P, (t + 1) * P)
                zt2 = tailp.tile([P, D], mybir.dt.float32, tag="zt2")
                nc.sync.dma_start(out=zt2, in_=z_d[tok, :])
                zq = tailp.tile([P, D], mybir.dt.float32, tag="zq")
                nc.gpsimd.indirect_dma_start(
                    out=zq, out_offset=None, in_=emb_d,
                    in_offset=bass.IndirectOffsetOnAxis(ap=w8all[:, t:t + 1], axis=0))
                diff = work.tile([P, D], mybir.dt.float32, tag="diff")
                nc.vector.tensor_sub(out=diff, in0=zq, in1=zt2)
                st = work.tile([P, D], mybir.dt.float32, tag="st")
                nc.vector.tensor_add(out=st, in0=zt2, in1=diff)
                nc.sync.dma_start(out=zqst_d[tok, :], in_=st)

            def body():
                for t in range(NTILES):
                    tok = slice(t * P, (t + 1) * P)
                    if mm_mode == "f32":
                        ztile = work.tile([P, D], mybir.dt.float32, tag="ztile")
                        nc.sync.dma_start(out=ztile, in_=z_d[tok, :])
                        ztT = work.tile([P, 2, P], mybir.dt.float32, tag="ztT")
                        nc.sync.dma_start(
                            out=ztT, in_=zT_d[:, tok].rearrange("(h p) m -> p h m", p=P))
                    elif t in pre:
                        ztile, zhT, zlT = pre[t]
                    else:
                        ztile = work.tile([P, D], mybir.dt.float32, tag="ztile")
                        nc.sync.dma_start(out=ztile, in_=z_d[tok, :])
                        zhT = work.tile([P, 2, P], mybir.dt.float16, tag="zhT")
                        nc.sync.dma_start(
                            out=zhT, in_=zh_d[:, tok].rearrange("(h p) m -> p h m", p=P))
                        zlT = work.tile([P, 2, P], mybir.dt.float16, tag="zlT")
                        nc.sync.dma_start(
                            out=zlT, in_=zl_d[:, tok].rearrange("(h p) m -> p h m", p=P))

                    # A = sum(z^2) per token
                    sq = work.tile([P, D], mybir.dt.float32, tag="sq")
                    Atile = small.tile([P, 1], mybir.dt.float32, tag="A")
                    nc.scalar.activation(out=sq, in_=ztile,
                                         func=mybir.ActivationFunctionType.Square,
                                         accum_out=Atile)

                    dtile = dpool.tile([P, K], mybir.dt.float32, tag="d")
                    if mm_mode == "f32":
                        passes = [(ztT, [et])]
                    else:
                        passes = [(zhT, ehs), (zlT, ehs), (zhT, els)]
                    # weight-major over groups of 8 chunks: each (w, h) pass
                    # loads weights once and streams 8 same-weight matmuls
                    GRP = 4
                    for g in range(NCH // GRP):
                        pss = []
                        for j in range(GRP):
                            ps_j = psum_pool.tile([P, CHUNK], mybir.dt.float32,
                                                  name=f"ps{j}", tag=f"ps{j}")
                            pss.append(ps_j)
                        np_ = len(passes)
                        for i, (w, segs) in enumerate(passes):
                            for h in range(2):
                                for j in range(GRP):
                                    c = g * GRP + j
                                    if len(segs) == 1:
                                        e, cc = segs[0], c
                                    else:
                                        e, cc = segs[c // GRP], c % GRP
                                    cs = slice(cc * CHUNK, (cc + 1) * CHUNK)
                                    nc.tensor.matmul(
                                        pss[j], lhsT=w[:, h, :], rhs=e[:, h, cs],
                                        start=(i == 0 and h == 0),
                                        stop=(i == np_ - 1 and h == 1))
                        for j in range(GRP):
                            c = g * GRP + j
                            cs = slice(c * CHUNK, (c + 1) * CHUNK)
                            # d = Relu(A - 2C); Relu is identity: d >> 0
                            nc.scalar.activation(
                                out=dtile[:, cs], in_=pss[j],
                                func=mybir.ActivationFunctionType.Relu,
                                bias=Atile, scale=d_scale)

                    # gmin doubles as the loss partial: d_min == ||z-e_idx||^2
                    # (the ||e||^2 term is absorbed by fp32 rounding, ~5e-9 rel)
                    gmin = lp_all[:, t:t + 1]
                    nc.vector.tensor_reduce(out=gmin, in_=dtile,
                                            axis=mybir.AxisListType.X,
                                            op=mybir.AluOpType.min)
                    gmin8 = small.tile([P, 8], mybir.dt.float32, tag="gmin8")
                    nc.vector.tensor_copy(out=gmin8, in_=gmin.to_broadcast([P, 8]))
                    w8 = small.tile([P, 8], mybir.dt.uint32, tag="w8")
                    nc.vector.max_index(out=w8, in_max=gmin8, in_values=dtile)
                    nc.vector.tensor_copy(out=w8all[:, t:t + 1], in_=w8[:, 0:1])
                    nc.sync.dma_start(
                        out=idx_d[tok].rearrange("(p one) -> p one", one=1),
                        in_=w8[:, 0:1].bitcast(mybir.dt.int32))
                    if t >= 2:
                        tail_work(t - 2)
                for t in range(max(0, NTILES - 2), NTILES):
                    tail_work(t)



            if reps == 1:
                body()
            else:
                with tc.For_i(0, reps, 1):
                    body()

            nc.sync.dma_start(out=loss_d, in_=lp_all)

    nc.compile()
    return nc


def _in_maps(z, emb, mm_mode=None):
    mm_mode = mm_mode or MM_MODE
    zf = np.ascontiguousarray(z.reshape(NTOK, D))
    emb_c = np.ascontiguousarray(emb)
    in_maps = []
    if mm_mode == "f32":
        embT2 = np.ascontiguousarray((2.0 * emb).T)
        for c in range(NCORES):
            zs = np.ascontiguousarray(zf[c * SHARD:(c + 1) * SHARD])
            in_maps.append({
                "z": zs,
                "zt": np.ascontiguousarray(zs.T),
                "embt2": embT2,
                "emb": emb_c,
            })
        return in_maps

    import ml_dtypes
    bf16 = ml_dtypes.bfloat16
    # emb side: es = emb.T * 2^15 (2x fold + fp16-range shift);
    # eh = fp16(es) [11-bit head], el = bf16(es - eh) [cross term]
    es = emb.T.astype(np.float32) * np.float32(2.0 ** ESCALE)
    eh = es.astype(np.float16)
    el = (es - eh.astype(np.float32)).astype(np.float16)
    eh = np.ascontiguousarray(eh)
    el = np.ascontiguousarray(el)
    # z side: zh = fp16(zT) [11-bit head], zl = fp16(zT - zh)
    # (PE computes fp16 subnormals exactly -- verified on HW)
    zT = zf.T.astype(np.float32)
    zh = zT.astype(np.float16)
    zl = (zT - zh.astype(np.float32)).astype(np.float16)
    for c in range(NCORES):
        tok = slice(c * SHARD, (c + 1) * SHARD)
        in_maps.append({
            "z": np.ascontiguousarray(zf[tok]),
            "zh": np.ascontiguousarray(zh[:, tok]),
            "zl": np.ascontiguousarray(zl[:, tok]),
            "eh": eh,
            "el": el,
            "emb": emb_c,
        })
    return in_maps


def _assemble(results):
    zqst = np.concatenate([r["zqst"] for r in results], axis=0).reshape(B, T, D)
    idx = np.concatenate([r["idx"] for r in results], axis=0).astype(np.int32)
    total = np.sum([r["losspart"].astype(np.float64).sum() for r in results])
    m = np.float32(total / (NTOK * D))
    loss = np.float32(m + np.float32(BETA) * m)
    return zqst, idx, loss


def _run(z, emb):
    if "nc" not in _cache:
        _cache["nc"] = _build()
    res = bass_utils.run_bass_kernel_spmd(
        _cache["nc"], in_maps=_in_maps(z, emb), core_ids=list(range(NCORES)))
    return _assemble(res.results)


def kernel(z, emb):
    return _run(np.asarray(z), np.asarray(emb))
